# revision 10
# baseline (speedup 1.0000x reference)
"""Trainium2 Bass kernel for nn_Attention_21878563405851.

Module: kv = x1 @ W_qk (k,v split); q = x2 @ W_v; 8-head attention
(dim_head=64); out @ W_out + b_out.  B=2, N=2048, DIM=512.

Sharding over 8 NeuronCores: core c -> batch b=c//4, query slice
qs = (c%4)*512 .. +512.  ZERO collectives: each core duplicates the
k/v projection for its whole batch locally (cheaper than the 20-40us
AllGathers a head-sharded variant needs) and computes its own
512-query slice of the output end to end.

Per core:
  1. load x1[b]^T (full, for k/v), x2[b, qs]^T, and weights as SBUF
     images.  Loads are split/ordered so the first k-projection matmul
     gates on only ~640KB (wk set0 + x1 chunk0); W_out/bias loads are
     deferred to overlap attention,
  2. k/q projections produce fp8(e4m3) kT32/qT32 tiles in DoubleRow
     layout ([32-partition quads, d-half pairs]); v projection lands
     directly in key-major layout (v_ext [128 keys, 8*(64+ones)]) so
     no PE transposes are needed; the softmax denominator falls out of
     attnv via the ones column,
  3. per head: per key-tile dots^T = k @ q^T as ONE fp8 DoubleRow
     matmul (d=64 contraction split 32x2, half PE cost); per key-tile
     -pair one [128,1024] exp on ACT -> bf16 e tile -> attnv (bf16,
     error-critical) accumulated into [65,512] PSUM.  Software-
     pipelined (dots of pair p+1 before attnv of pair p) so the
     in-order PE queue never waits on ACT.  k/q projection sets for
     later heads are interleaved between heads to fill PE slack under
     ACT's exp chain,
  4. per-head normalization (reciprocal of the ones-row + DMA
     broadcast + DVE multiply) overlaps the next head,
  5. y^T Mtiles = W_out^T @ attn_out^T with the bias folded in as a
     ones-row matmul, DMA'd straight from PSUM as yT [512,512] f32;
     the host transposes and concatenates.

fp8 is used ONLY for dots (k/q operands): measured end-to-end rel err
1.34e-2 vs the 2e-2 gate (numpy model); e/v/attnv and all other
matmuls stay bf16 (each would add 2-3e-2 alone).
"""

import sys

for _p in ("/opt/trn_rl_repo", "/root/.axon_site/_ro/trn_rl_repo"):
    if _p not in sys.path:
        sys.path.insert(0, _p)

import numpy as np
import ml_dtypes

import concourse.bass as bass
import concourse.mybir as mybir
from concourse import tile
from concourse.bacc import Bacc

B, N, DIM = 2, 2048, 512
HEADS, DH = 8, 64
INNER = HEADS * DH
SCALE = DH ** -0.5
NCORES = 8
NQ = 512             # queries per core
NKT = N // 128       # key tiles (16)
NPAIR = NKT // 2     # key-tile pairs (8)
NC_CHUNKS = DIM // 128
NSETS = 4            # projection sets: (quad, d-half) for k/q
NQUAD = 2            # head quads (4 heads each)
VW = 66              # v_ext per-head stride (64 v cols + ones + pad)
VKT = HEADS * VW     # v_ext per-key-tile stride (528)

BF16 = mybir.dt.bfloat16
F32 = mybir.dt.float32
FP8 = mybir.dt.float8e4


def build_program():
    nc = Bacc(None, num_devices=NCORES)

    # ---- external I/O (per core) ----
    # x images are pre-arranged host-side in SBUF chunk layout:
    # partition p holds row (chunk*128 + p), chunks along the free dim.
    x1T = nc.dram_tensor("x1T", [128, NC_CHUNKS * N], BF16, kind="ExternalInput")
    x2T = nc.dram_tensor("x2T", [128, NC_CHUNKS * NQ], BF16, kind="ExternalInput")
    # wk/wq col block (quad g, half t, chunk c): [128, 128], col m ->
    # W col (g*4 + m//32)*64 + t*32 + m%32.  wv [128, (chunk) 512],
    # wo [128, (mtile chunk) 128], bo [1, 512] bf16 (ones-row bias).
    wk = nc.dram_tensor("wk", [128, NSETS * NC_CHUNKS * 128], BF16, kind="ExternalInput")
    wq = nc.dram_tensor("wq", [128, NSETS * NC_CHUNKS * 128], BF16, kind="ExternalInput")
    wv = nc.dram_tensor("wv", [128, NC_CHUNKS * 512], BF16, kind="ExternalInput")
    wo = nc.dram_tensor("wo", [128, NC_CHUNKS * NC_CHUNKS * 128], BF16, kind="ExternalInput")
    bo = nc.dram_tensor("bo", [1, DIM], BF16, kind="ExternalInput")
    yT = nc.dram_tensor("yT", [DIM, NQ], F32, kind="ExternalOutput")

    with tile.TileContext(nc) as tc:
        with (
            tc.tile_pool(name="xin", bufs=1) as xin,
            tc.tile_pool(name="wts", bufs=1) as wts,
            tc.tile_pool(name="kq", bufs=1) as kqp,
            tc.tile_pool(name="vext", bufs=1) as vextp,
            tc.tile_pool(name="et", bufs=4) as etp,
            tc.tile_pool(name="norm", bufs=2) as normp,
            tc.tile_pool(name="outp", bufs=1) as outp,
            tc.tile_pool(name="yout", bufs=2) as youtp,
            # PSUM: ps_pair = 3 rotating [128,1024] f32 slots (6 banks)
            # for proj / dots / final-y; ps_acc = 2 rotating [65,512]
            # attnv accumulators (2 banks).
            tc.tile_pool(name="ps_pair", bufs=3, space="PSUM") as psp,
            tc.tile_pool(name="ps_acc", bufs=2, space="PSUM") as ps_acc,
            tc.tile_pool(name="dram", bufs=1, space="DRAM") as dramp,
        ):
            # ---- priority loads: first k-proj gates on wk set0 + x1 chunk0
            wk_s = wts.tile([128, NSETS * NC_CHUNKS * 128], BF16, name="wk_s")
            SETW = NC_CHUNKS * 128
            nc.gpsimd.dma_start(wk_s[:, 0:SETW], wk[:, 0:SETW])
            x1_s = xin.tile([128, NC_CHUNKS * N], BF16, name="x1_s")
            for c in range(NC_CHUNKS):
                nc.sync.dma_start(
                    x1_s[:, c * N:(c + 1) * N], x1T[:, c * N:(c + 1) * N]
                )
            wq_s = wts.tile([128, NSETS * NC_CHUNKS * 128], BF16, name="wq_s")
            nc.gpsimd.dma_start(wq_s[:, 0:SETW], wq[:, 0:SETW])
            x2_s = xin.tile([128, NC_CHUNKS * NQ], BF16, name="x2_s")
            nc.scalar.dma_start(x2_s[:], x2T[:])
            wv_s = wts.tile([128, NC_CHUNKS * 512], BF16, name="wv_s")
            nc.scalar.dma_start(wv_s[:], wv[:])
            nc.gpsimd.dma_start(wk_s[:, SETW:], wk[:, SETW:])
            nc.gpsimd.dma_start(wq_s[:, SETW:], wq[:, SETW:])

            # persistent SBUF tensors
            # kT32[p]: [64, 2*N] fp8 for head pair (2p, 2p+1); partition
            # = 32*(h%2) + d%32 (AP bases are limited to {0,32,64}),
            # cols = (d//32)*N + key.  qT32[p]: [64, 2*NQ] fp8.
            kT32 = [
                kqp.tile([64, 2 * N], FP8, name=f"kT{p}") for p in range(4)
            ]
            qT32 = [
                kqp.tile([64, 2 * NQ], FP8, name=f"qT{p}") for p in range(4)
            ]
            v_ext = vextp.tile([128, NKT * VKT], BF16, name="v_ext")
            outT = outp.tile([128, NC_CHUNKS * NQ], BF16, name="outT")
            ones_s = wts.tile([1, NQ], BF16, name="ones_s")
            nc.vector.memset(ones_s[:], 1.0)
            bo_s = wts.tile([1, DIM], BF16, name="bo_s")

            def k_proj(si, drain_eng=None):
                """k half t=si%2 of head pairs (2g, 2g+1), g=si//2."""
                g, t = si // 2, si % 2
                ts = [
                    psp.tile([128, 1024], F32, name=f"kp{si}{i}", tag="ps")
                    for i in range(2)
                ]
                for c in range(NC_CHUNKS):
                    for p in range(4):
                        nc.tensor.matmul(
                            ts[p // 2][:, (p % 2) * 512:(p % 2) * 512 + 512],
                            wk_s[:, (si * NC_CHUNKS + c) * 128:(si * NC_CHUNKS + c + 1) * 128],
                            x1_s[:, c * N + p * 512: c * N + (p + 1) * 512],
                            start=(c == 0),
                            stop=(c == NC_CHUNKS - 1),
                        )
                eng = drain_eng or nc.vector
                for i in range(2):
                    for half in range(2):
                        # psum rows 0:64 -> pair 2g, 64:128 -> pair 2g+1
                        eng.tensor_copy(
                            kT32[2 * g + half][:, t * N + i * 1024: t * N + (i + 1) * 1024],
                            ts[i][half * 64:(half + 1) * 64, :],
                        )

            def q_proj(g):
                """qT32 pairs 2g, 2g+1 (both halves: sets 2g, 2g+1)."""
                ts = psp.tile([128, 1024], F32, name=f"qp{g}", tag="ps")
                for t in range(2):
                    si = 2 * g + t
                    for c in range(NC_CHUNKS):
                        nc.tensor.matmul(
                            ts[:, t * 512:(t + 1) * 512],
                            wq_s[:, (si * NC_CHUNKS + c) * 128:(si * NC_CHUNKS + c + 1) * 128],
                            x2_s[:, c * NQ:(c + 1) * NQ],
                            start=(c == 0),
                            stop=(c == NC_CHUNKS - 1),
                        )
                for half in range(2):
                    nc.vector.tensor_copy(
                        qT32[2 * g + half][:],
                        ts[half * 64:(half + 1) * 64, :],
                    )

            def v_proj():
                """v_ext in key-major layout: per kt, all 8 heads' v
                columns + a ones column per head (softmax denominator)."""
                for pr in range(NPAIR):
                    ts = psp.tile([128, 1024], F32, name=f"vp{pr}", tag="ps")
                    for half in range(2):
                        kt = 2 * pr + half
                        for c in range(NC_CHUNKS):
                            nc.tensor.matmul(
                                ts[:, half * 512:(half + 1) * 512],
                                x1_s[:, c * N + kt * 128: c * N + (kt + 1) * 128],
                                wv_s[:, c * 512:(c + 1) * 512],
                                start=(c == 0),
                                stop=(c == NC_CHUNKS - 1),
                            )
                    # strided drain: [128, 2, 8, 64] -> v_ext stride VW
                    nc.vector.tensor_copy(
                        v_ext[:, 2 * pr * VKT:(2 * pr + 2) * VKT].rearrange(
                            "p (two h w) -> p two h w", two=2, h=HEADS, w=VW
                        )[:, :, :, 0:DH],
                        ts[:].rearrange(
                            "p (two h w) -> p two h w", two=2, h=HEADS, w=DH
                        ),
                    )
                # ones columns: [128, kt, h, 1] at offset h*VW + 64
                nc.vector.memset(
                    v_ext[:].rearrange(
                        "p (kt h w) -> p kt h w", kt=NKT, h=HEADS, w=VW
                    )[:, :, :, DH:DH + 1],
                    1.0,
                )

            # ---- projections needed before attention starts ----
            k_proj(0)
            k_proj(1)
            q_proj(0)
            v_proj()
            # deferred loads (queue slots after the priority stream)
            wo_s = wts.tile([128, NC_CHUNKS * NC_CHUNKS * 128], BF16, name="wo_s")
            nc.gpsimd.dma_start(wo_s[:], wo[:])
            nc.gpsimd.dma_start(bo_s[:], bo[:])

            # ---- attention, key-pair software pipeline across heads ----
            # After head h (h<3), remaining k/q projection sets are
            # interleaved to fill PE slack under ACT's exp chain.
            pending = None  # (head, pair, e_tile)
            accs = {}

            def emit_attnv(h, pr, e_t):
                acc = accs[h]
                for half in range(2):
                    kt = 2 * pr + half
                    nc.tensor.matmul(
                        acc[:, 0:512],
                        v_ext[:, kt * VKT + (h * VW): kt * VKT + (h * VW) + 65],
                        e_t[:, half * 512:(half + 1) * 512],
                        start=(kt == 0),
                        stop=(kt == NKT - 1),
                    )
                if pr == NPAIR - 1:
                    emit_norm(h)

            def emit_norm(h):
                """acc[h] rows 0:64 / row 64 -> outT Mtile h//2."""
                acc = accs[h]
                s_s = normp.tile([1, NQ], F32, name="s_s", tag="s1")
                r_s = normp.tile([1, NQ], F32, name="r_s", tag="s2")
                rb_s = normp.tile([64, NQ], F32, name="rb_s", tag="rb")
                nc.vector.tensor_copy(s_s[:], acc[64:65, :])
                nc.vector.reciprocal_approx_fast(r_s[:], s_s[:])
                r_dram = dramp.tile([1, NQ], F32, name="r_dram", tag="r_dram", bufs=2)
                nc.sync.dma_start(r_dram[:], r_s[:])
                nc.sync.dma_start(rb_s[:], r_dram[0:1, :].broadcast_to([64, NQ]))
                m, lo = h // 2, (h % 2) * 64
                nc.vector.tensor_mul(
                    outT[lo:lo + 64, m * NQ:(m + 1) * NQ], acc[0:64, :], rb_s[:]
                )

            for h in range(HEADS):
                pr2, base = h // 2, 32 * (h % 2)
                kap = kT32[pr2][base:base + 32, :].rearrange(
                    "p (t n) -> p t n", t=2
                )
                qap = qT32[pr2][base:base + 32, :].rearrange(
                    "p (t n) -> p t n", t=2
                )
                accs[h] = ps_acc.tile([65, NQ], F32, name=f"acc{h}", tag="acc")
                for pr in range(NPAIR):
                    dt = psp.tile([128, 1024], F32, name="dt", tag="ps")
                    for half in range(2):
                        kt = 2 * pr + half
                        nc.tensor.matmul(
                            dt[:, half * 512:(half + 1) * 512],
                            kap[:, :, kt * 128:(kt + 1) * 128],
                            qap[:],
                            perf_mode=mybir.MatmulPerfMode.DoubleRow,
                        )
                    e_t = etp.tile([128, 1024], BF16, name="e_t", tag="e")
                    nc.scalar.activation(
                        e_t[:], dt[:],
                        mybir.ActivationFunctionType.Exp, scale=SCALE,
                    )
                    if pending is not None:
                        emit_attnv(*pending)
                    pending = (h, pr, e_t)
                # interleave remaining projections under ACT slack
                if h == 0:
                    k_proj(2)
                elif h == 1:
                    k_proj(3)
                elif h == 2:
                    q_proj(1)
            emit_attnv(*pending)

            # ---- final projection: yT Mtiles, bias via ones-row ----
            y_ps = [
                psp.tile([128, 1024], F32, name=f"y{i}", tag="ps")
                for i in range(2)
            ]
            for c in range(NC_CHUNKS):
                for m in range(NC_CHUNKS):
                    nc.tensor.matmul(
                        y_ps[m // 2][:, (m % 2) * 512:(m % 2) * 512 + 512],
                        wo_s[:, (m * NC_CHUNKS + c) * 128:(m * NC_CHUNKS + c + 1) * 128],
                        outT[:, c * NQ:(c + 1) * NQ],
                        start=(c == 0),
                        stop=False,
                    )
            for m in range(NC_CHUNKS):
                nc.tensor.matmul(
                    y_ps[m // 2][:, (m % 2) * 512:(m % 2) * 512 + 512],
                    bo_s[:, m * 128:(m + 1) * 128],
                    ones_s[:],
                    start=False,
                    stop=True,
                )
                y_out = youtp.tile([128, NQ], F32, name="y_out", tag="y")
                nc.vector.tensor_copy(
                    y_out[:],
                    y_ps[m // 2][:, (m % 2) * 512:(m % 2) * 512 + 512],
                )
                nc.scalar.dma_start(yT[m * 128:(m + 1) * 128, :], y_out[:])

    nc.finalize()
    return nc


_NC_CACHE = None


def _get_program():
    global _NC_CACHE
    if _NC_CACHE is None:
        _NC_CACHE = build_program()
    return _NC_CACHE


def _img_chunks(a):
    """[DIM, cols] -> SBUF chunk image [128, NC_CHUNKS*cols]."""
    cols = a.shape[1]
    return np.ascontiguousarray(
        a.reshape(NC_CHUNKS, 128, cols).transpose(1, 0, 2).reshape(128, -1)
    )


def _kq_img(W):
    """[DIM, 512] k- or q-weights -> DoubleRow-layout image.

    Col block (si=(quad,half), chunk c) = [128,128]; block col m maps to
    W col (quad*4 + m//32)*64 + half*32 + (m%32)."""
    blocks = []
    for si in range(NSETS):
        g, t = si // 2, si % 2
        cols = np.array(
            [(g * 4 + m // 32) * DH + t * 32 + (m % 32) for m in range(128)]
        )
        for c in range(NC_CHUNKS):
            blocks.append(W[c * 128:(c + 1) * 128, cols])
    return np.ascontiguousarray(np.stack(blocks, axis=1).reshape(128, -1))


def make_in_maps(x1, x2, W_qk, W_v, W_out, b_out):
    bf = ml_dtypes.bfloat16
    x1 = np.asarray(x1, np.float32)
    x2 = np.asarray(x2, np.float32)
    W_qk = np.asarray(W_qk, np.float32).astype(bf)
    W_v = np.asarray(W_v, np.float32).astype(bf)
    W_out = np.asarray(W_out, np.float32).astype(bf)
    b_out = np.asarray(b_out, np.float32)

    wk_img = _kq_img(W_qk[:, :INNER])
    wq_img = _kq_img(W_v)
    wv_img = _img_chunks(W_qk[:, INNER:])
    wo_img = np.ascontiguousarray(
        np.stack(
            [
                W_out[c * 128:(c + 1) * 128, m * 128:(m + 1) * 128]
                for m in range(NC_CHUNKS) for c in range(NC_CHUNKS)
            ], axis=1,
        ).reshape(128, -1)
    )
    bo_img = np.ascontiguousarray(b_out.reshape(1, DIM).astype(bf))

    x1T_img = [
        _img_chunks(np.ascontiguousarray(x1[b].T).astype(bf)) for b in range(B)
    ]

    in_maps = []
    for c in range(NCORES):
        b, qi = c // 4, c % 4
        x2T_img = _img_chunks(
            np.ascontiguousarray(x2[b, qi * NQ:(qi + 1) * NQ, :].T).astype(bf)
        )
        in_maps.append(
            {
                "x1T": x1T_img[b],
                "x2T": x2T_img,
                "wk": wk_img,
                "wq": wq_img,
                "wv": wv_img,
                "wo": wo_img,
                "bo": bo_img,
            }
        )
    return in_maps


def assemble_output(results):
    y = np.empty((B, N, DIM), np.float32)
    for c in range(NCORES):
        b, qi = c // 4, c % 4
        y[b, qi * NQ:(qi + 1) * NQ, :] = results[c]["yT"].T
    return y


def kernel(x1, x2, W_qk, W_v, W_out, b_out):
    from concourse.bass_utils import run_bass_kernel_spmd

    nc = _get_program()
    in_maps = make_in_maps(x1, x2, W_qk, W_v, W_out, b_out)
    res = run_bass_kernel_spmd(nc, in_maps, list(range(NCORES)))
    return assemble_output(res.results)


# revision 11
# speedup vs baseline: 1.1756x; 1.1756x over previous
"""Trainium2 Bass kernel for nn_Attention_21878563405851.

Module: kv = x1 @ W_qk (k,v split); q = x2 @ W_v; 8-head attention
(dim_head=64); out @ W_out + b_out.  B=2, N=2048, DIM=512.

Sharding over 8 NeuronCores: core c -> batch b=c//4, query slice
qs = (c%4)*512 .. +512.  ZERO collectives: each core duplicates the
k/v projection for its whole batch locally (cheaper than the 20-40us
AllGathers a head-sharded variant needs) and computes its own
512-query slice of the output end to end.

All matmuls are bf16 (fp8 DoubleRow dots was tried and measured
SLOWER: mixing fp8-DR and bf16 instructions drops the whole PE phase
from ~375ns/matmul to ~630ns/matmul, and the fp8 drains add ~12us of
DVE to the critical path; rel err also rises 3.9e-3 -> 1.35e-2).

Per core:
  1. load x1[b]^T (full, for k/v), x2[b, qs]^T, and weights as SBUF
     images.  Loads are split/ordered so the first k-projection matmul
     gates on only ~640KB (wk set0 + x1 chunk0); W_out/bias loads are
     deferred to overlap attention,
  2. k projection (head-pair sets, d-major kT tiles), q projection
     (per-set [128,512]), v projection directly in key-major layout
     (v_ext [128 keys, 8*(64+ones)]) so no PE transposes are needed;
     the softmax denominator falls out of attnv via the ones column.
     Only set 0 + v run before attention; k/q sets 1-3 are interleaved
     between attention heads to fill PE slack under ACT's exp chain,
  3. per head: per key-tile dots^T = k @ q^T ([128,512], K=64); per
     key-tile-pair one [128,1024] exp on ACT -> bf16 e tile -> attnv
     accumulated into [65,512] PSUM.  Software-pipelined (dots of pair
     p+1 emitted before attnv of pair p) so the in-order PE queue
     never waits on ACT,
  4. per-head normalization (reciprocal of the ones-row + DMA
     broadcast + DVE multiply) overlaps the next head,
  5. y^T Mtiles = W_out^T @ attn_out^T with the bias folded in as a
     ones-row matmul, staged through SBUF and DMA'd as yT [512,512]
     f32; the host transposes and concatenates.
"""

import sys

for _p in ("/opt/trn_rl_repo", "/root/.axon_site/_ro/trn_rl_repo"):
    if _p not in sys.path:
        sys.path.insert(0, _p)

import numpy as np
import ml_dtypes

import concourse.bass as bass
import concourse.mybir as mybir
from concourse import tile
from concourse.bacc import Bacc

B, N, DIM = 2, 2048, 512
HEADS, DH = 8, 64
INNER = HEADS * DH
SCALE = DH ** -0.5
NCORES = 8
NQ = 512             # queries per core
NKT = N // 128       # key tiles (16)
NPAIR = NKT // 2     # key-tile pairs (8)
NC_CHUNKS = DIM // 128
NSETS = 4            # head-pair sets (2 heads x 64 = 128 cols each)
VW = 66              # v_ext per-head stride (64 v cols + ones + pad)
VKT = HEADS * VW     # v_ext per-key-tile stride (528)

BF16 = mybir.dt.bfloat16
F32 = mybir.dt.float32


def build_program():
    nc = Bacc(None, num_devices=NCORES)

    # ---- external I/O (per core) ----
    # x images are pre-arranged host-side in SBUF chunk layout:
    # partition p holds row (chunk*128 + p), chunks along the free dim.
    x1T = nc.dram_tensor("x1T", [128, NC_CHUNKS * N], BF16, kind="ExternalInput")
    x2T = nc.dram_tensor("x2T", [128, NC_CHUNKS * NQ], BF16, kind="ExternalInput")
    # weight images (see make_in_maps): wk/wq [128, (set chunk) 128],
    # wv [128, (chunk) 512], wo [128, (mtile chunk) 128], bo [1, 512]
    wk = nc.dram_tensor("wk", [128, NSETS * NC_CHUNKS * 128], BF16, kind="ExternalInput")
    wq = nc.dram_tensor("wq", [128, NSETS * NC_CHUNKS * 128], BF16, kind="ExternalInput")
    wv = nc.dram_tensor("wv", [128, NC_CHUNKS * 512], BF16, kind="ExternalInput")
    wo = nc.dram_tensor("wo", [128, NC_CHUNKS * NC_CHUNKS * 128], BF16, kind="ExternalInput")
    bo = nc.dram_tensor("bo", [1, DIM], BF16, kind="ExternalInput")
    yT = nc.dram_tensor("yT", [DIM, NQ], F32, kind="ExternalOutput")

    with tile.TileContext(nc) as tc:
        with (
            tc.tile_pool(name="xin", bufs=1) as xin,
            tc.tile_pool(name="wts", bufs=1) as wts,
            tc.tile_pool(name="kq", bufs=1) as kqp,
            tc.tile_pool(name="vext", bufs=1) as vextp,
            tc.tile_pool(name="et", bufs=4) as etp,
            tc.tile_pool(name="norm", bufs=2) as normp,
            tc.tile_pool(name="outp", bufs=1) as outp,
            tc.tile_pool(name="yout", bufs=2) as youtp,
            # PSUM: ps_pair = 3 rotating [128,1024] f32 slots (6 banks)
            # for proj / dots / final-y; ps_acc = 2 rotating [65,512]
            # attnv accumulators (2 banks).
            tc.tile_pool(name="ps_pair", bufs=3, space="PSUM") as psp,
            tc.tile_pool(name="ps_acc", bufs=2, space="PSUM") as ps_acc,
            tc.tile_pool(name="dram", bufs=1, space="DRAM") as dramp,
        ):
            # ---- priority loads: first k-proj gates on wk set0 + x1 chunk0
            wk_s = wts.tile([128, NSETS * NC_CHUNKS * 128], BF16, name="wk_s")
            SETW = NC_CHUNKS * 128
            nc.gpsimd.dma_start(wk_s[:, 0:SETW], wk[:, 0:SETW])
            x1_s = xin.tile([128, NC_CHUNKS * N], BF16, name="x1_s")
            for c in range(NC_CHUNKS):
                nc.sync.dma_start(
                    x1_s[:, c * N:(c + 1) * N], x1T[:, c * N:(c + 1) * N]
                )
            wq_s = wts.tile([128, NSETS * NC_CHUNKS * 128], BF16, name="wq_s")
            nc.gpsimd.dma_start(wq_s[:, 0:SETW], wq[:, 0:SETW])
            x2_s = xin.tile([128, NC_CHUNKS * NQ], BF16, name="x2_s")
            nc.scalar.dma_start(x2_s[:], x2T[:])
            wv_s = wts.tile([128, NC_CHUNKS * 512], BF16, name="wv_s")
            nc.scalar.dma_start(wv_s[:], wv[:])
            nc.gpsimd.dma_start(wk_s[:, SETW:], wk[:, SETW:])
            nc.gpsimd.dma_start(wq_s[:, SETW:], wq[:, SETW:])

            # persistent SBUF tensors
            kT = [
                kqp.tile([128, N], BF16, name=f"kT{s}") for s in range(NSETS)
            ]  # set s: heads 2s (rows 0:64), 2s+1 (rows 64:128), d-major
            qT = [
                kqp.tile([128, NQ], BF16, name=f"qT{s}") for s in range(NSETS)
            ]
            v_ext = vextp.tile([128, NKT * VKT], BF16, name="v_ext")
            outT = outp.tile([128, NC_CHUNKS * NQ], BF16, name="outT")
            ones_s = wts.tile([1, NQ], BF16, name="ones_s")
            nc.vector.memset(ones_s[:], 1.0)
            bo_s = wts.tile([1, DIM], BF16, name="bo_s")

            def k_proj(s):
                """kT[s] <- (W_qk k-cols for heads 2s,2s+1)^T @ x1[b]^T."""
                ts = [
                    psp.tile([128, 1024], F32, name=f"kp{s}{i}", tag="ps")
                    for i in range(2)
                ]
                for c in range(NC_CHUNKS):
                    for p in range(4):
                        nc.tensor.matmul(
                            ts[p // 2][:, (p % 2) * 512:(p % 2) * 512 + 512],
                            wk_s[:, (s * NC_CHUNKS + c) * 128:(s * NC_CHUNKS + c + 1) * 128],
                            x1_s[:, c * N + p * 512: c * N + (p + 1) * 512],
                            start=(c == 0),
                            stop=(c == NC_CHUNKS - 1),
                        )
                for i in range(2):
                    nc.vector.tensor_copy(
                        kT[s][:, i * 1024:(i + 1) * 1024], ts[i][:]
                    )

            def q_proj(s):
                ts = psp.tile([128, 1024], F32, name=f"qp{s}", tag="ps")
                for c in range(NC_CHUNKS):
                    nc.tensor.matmul(
                        ts[:, 0:512],
                        wq_s[:, (s * NC_CHUNKS + c) * 128:(s * NC_CHUNKS + c + 1) * 128],
                        x2_s[:, c * NQ:(c + 1) * NQ],
                        start=(c == 0),
                        stop=(c == NC_CHUNKS - 1),
                    )
                nc.vector.tensor_copy(qT[s][:], ts[:, 0:512])

            def v_proj():
                """v_ext in key-major layout: per kt, all 8 heads' v
                columns + a ones column per head (softmax denominator)."""
                for pr in range(NPAIR):
                    ts = psp.tile([128, 1024], F32, name=f"vp{pr}", tag="ps")
                    for half in range(2):
                        kt = 2 * pr + half
                        for c in range(NC_CHUNKS):
                            nc.tensor.matmul(
                                ts[:, half * 512:(half + 1) * 512],
                                x1_s[:, c * N + kt * 128: c * N + (kt + 1) * 128],
                                wv_s[:, c * 512:(c + 1) * 512],
                                start=(c == 0),
                                stop=(c == NC_CHUNKS - 1),
                            )
                    # strided drain: [128, 2, 8, 64] -> v_ext stride VW
                    nc.vector.tensor_copy(
                        v_ext[:, 2 * pr * VKT:(2 * pr + 2) * VKT].rearrange(
                            "p (two h w) -> p two h w", two=2, h=HEADS, w=VW
                        )[:, :, :, 0:DH],
                        ts[:].rearrange(
                            "p (two h w) -> p two h w", two=2, h=HEADS, w=DH
                        ),
                    )
                # ones columns: [128, kt, h, 1] at offset h*VW + 64
                nc.vector.memset(
                    v_ext[:].rearrange(
                        "p (kt h w) -> p kt h w", kt=NKT, h=HEADS, w=VW
                    )[:, :, :, DH:DH + 1],
                    1.0,
                )

            # ---- projections needed before attention starts ----
            k_proj(0)
            q_proj(0)
            v_proj()
            # deferred loads (queue slots after the priority stream)
            wo_s = wts.tile([128, NC_CHUNKS * NC_CHUNKS * 128], BF16, name="wo_s")
            nc.gpsimd.dma_start(wo_s[:], wo[:])
            nc.gpsimd.dma_start(bo_s[:], bo[:])

            # ---- attention, key-pair software pipeline across heads ----
            pending = None  # (head, pair, e_tile)
            accs = {}

            def emit_attnv(h, pr, e_t):
                acc = accs[h]
                for half in range(2):
                    kt = 2 * pr + half
                    nc.tensor.matmul(
                        acc[:, 0:512],
                        v_ext[:, kt * VKT + (h * VW): kt * VKT + (h * VW) + 65],
                        e_t[:, half * 512:(half + 1) * 512],
                        start=(kt == 0),
                        stop=(kt == NKT - 1),
                    )
                if pr == NPAIR - 1:
                    emit_norm(h)

            def emit_norm(h):
                """acc[h] rows 0:64 / row 64 -> outT Mtile h//2."""
                acc = accs[h]
                s_s = normp.tile([1, NQ], F32, name="s_s", tag="s1")
                r_s = normp.tile([1, NQ], F32, name="r_s", tag="s2")
                rb_s = normp.tile([64, NQ], F32, name="rb_s", tag="rb")
                nc.vector.tensor_copy(s_s[:], acc[64:65, :])
                nc.vector.reciprocal_approx_fast(r_s[:], s_s[:])
                r_dram = dramp.tile([1, NQ], F32, name="r_dram", tag="r_dram", bufs=2)
                nc.sync.dma_start(r_dram[:], r_s[:])
                nc.sync.dma_start(rb_s[:], r_dram[0:1, :].broadcast_to([64, NQ]))
                m, lo = h // 2, (h % 2) * 64
                nc.vector.tensor_mul(
                    outT[lo:lo + 64, m * NQ:(m + 1) * NQ], acc[0:64, :], rb_s[:]
                )

            for h in range(HEADS):
                s, lo = h // 2, (h % 2) * 64
                accs[h] = ps_acc.tile([65, NQ], F32, name=f"acc{h}", tag="acc")
                for pr in range(NPAIR):
                    dt = psp.tile([128, 1024], F32, name="dt", tag="ps")
                    for half in range(2):
                        kt = 2 * pr + half
                        nc.tensor.matmul(
                            dt[:, half * 512:(half + 1) * 512],
                            kT[s][lo:lo + 64, kt * 128:(kt + 1) * 128],
                            qT[s][lo:lo + 64, :],
                        )
                    e_t = etp.tile([128, 1024], BF16, name="e_t", tag="e")
                    nc.scalar.activation(
                        e_t[:], dt[:],
                        mybir.ActivationFunctionType.Exp, scale=SCALE,
                    )
                    if pending is not None:
                        emit_attnv(*pending)
                    pending = (h, pr, e_t)
                # interleave remaining projections under ACT slack
                if h < NSETS - 1:
                    k_proj(h + 1)
                    q_proj(h + 1)
            emit_attnv(*pending)

            # ---- final projection: yT Mtiles, bias via ones-row ----
            y_ps = [
                psp.tile([128, 1024], F32, name=f"y{i}", tag="ps")
                for i in range(2)
            ]
            for c in range(NC_CHUNKS):
                for m in range(NC_CHUNKS):
                    nc.tensor.matmul(
                        y_ps[m // 2][:, (m % 2) * 512:(m % 2) * 512 + 512],
                        wo_s[:, (m * NC_CHUNKS + c) * 128:(m * NC_CHUNKS + c + 1) * 128],
                        outT[:, c * NQ:(c + 1) * NQ],
                        start=(c == 0),
                        stop=False,
                    )
            for m in range(NC_CHUNKS):
                nc.tensor.matmul(
                    y_ps[m // 2][:, (m % 2) * 512:(m % 2) * 512 + 512],
                    bo_s[:, m * 128:(m + 1) * 128],
                    ones_s[:],
                    start=False,
                    stop=True,
                )
                y_out = youtp.tile([128, NQ], F32, name="y_out", tag="y")
                nc.vector.tensor_copy(
                    y_out[:],
                    y_ps[m // 2][:, (m % 2) * 512:(m % 2) * 512 + 512],
                )
                nc.scalar.dma_start(yT[m * 128:(m + 1) * 128, :], y_out[:])

    nc.finalize()
    return nc


_NC_CACHE = None


def _get_program():
    global _NC_CACHE
    if _NC_CACHE is None:
        _NC_CACHE = build_program()
    return _NC_CACHE


def _img_chunks(a):
    """[DIM, cols] -> SBUF chunk image [128, NC_CHUNKS*cols]."""
    cols = a.shape[1]
    return np.ascontiguousarray(
        a.reshape(NC_CHUNKS, 128, cols).transpose(1, 0, 2).reshape(128, -1)
    )


def make_in_maps(x1, x2, W_qk, W_v, W_out, b_out):
    bf = ml_dtypes.bfloat16
    x1 = np.asarray(x1, np.float32)
    x2 = np.asarray(x2, np.float32)
    W_qk = np.asarray(W_qk, np.float32).astype(bf)
    W_v = np.asarray(W_v, np.float32).astype(bf)
    W_out = np.asarray(W_out, np.float32).astype(bf)
    b_out = np.asarray(b_out, np.float32)

    # wk/wq images: col block (set*NC_CHUNKS + c) = W[chunk c, heads 2s,2s+1]
    wk_img = np.ascontiguousarray(
        np.stack(
            [
                W_qk[c * 128:(c + 1) * 128, s * 128:(s + 1) * 128]
                for s in range(NSETS) for c in range(NC_CHUNKS)
            ], axis=1,
        ).reshape(128, -1)
    )
    wq_img = np.ascontiguousarray(
        np.stack(
            [
                W_v[c * 128:(c + 1) * 128, s * 128:(s + 1) * 128]
                for s in range(NSETS) for c in range(NC_CHUNKS)
            ], axis=1,
        ).reshape(128, -1)
    )
    wv_img = _img_chunks(W_qk[:, INNER:])
    wo_img = np.ascontiguousarray(
        np.stack(
            [
                W_out[c * 128:(c + 1) * 128, m * 128:(m + 1) * 128]
                for m in range(NC_CHUNKS) for c in range(NC_CHUNKS)
            ], axis=1,
        ).reshape(128, -1)
    )
    bo_img = np.ascontiguousarray(b_out.reshape(1, DIM).astype(bf))

    x1T_img = [
        _img_chunks(np.ascontiguousarray(x1[b].T).astype(bf)) for b in range(B)
    ]

    in_maps = []
    for c in range(NCORES):
        b, qi = c // 4, c % 4
        x2T_img = _img_chunks(
            np.ascontiguousarray(x2[b, qi * NQ:(qi + 1) * NQ, :].T).astype(bf)
        )
        in_maps.append(
            {
                "x1T": x1T_img[b],
                "x2T": x2T_img,
                "wk": wk_img,
                "wq": wq_img,
                "wv": wv_img,
                "wo": wo_img,
                "bo": bo_img,
            }
        )
    return in_maps


def assemble_output(results):
    y = np.empty((B, N, DIM), np.float32)
    for c in range(NCORES):
        b, qi = c // 4, c % 4
        y[b, qi * NQ:(qi + 1) * NQ, :] = results[c]["yT"].T
    return y


def kernel(x1, x2, W_qk, W_v, W_out, b_out):
    from concourse.bass_utils import run_bass_kernel_spmd

    nc = _get_program()
    in_maps = make_in_maps(x1, x2, W_qk, W_v, W_out, b_out)
    res = run_bass_kernel_spmd(nc, in_maps, list(range(NCORES)))
    return assemble_output(res.results)
